# revision 17
# baseline (speedup 1.0000x reference)
"""GAT-style attention kernel for Trainium2, 8-core row-parallel.

Reference computation:
    h = x @ W; s1 = h @ a1; s2 = h @ a2
    e[i,j] = leaky_relu(s1[j] + s2[i], 0.2); masked by adj; row-softmax; @ h

Key algebraic trick: with the column rescale w~ = w / exp(0.2*s2[i]),
    w~[j,i] = adj[i,j] * max(exp(s1[j] + 0.8*s2[i]), exp(0.2*s1[j]))
and the rescale cancels in the softmax normalization:
    out[i,:] = (sum_j w~[j,i] h[j,:]) / (sum_j w~[j,i]).
The exp is separable: exp(s1[j] + 0.8*s2[i]) = exp(s1[j]) * exp(0.8*s2[i]),
so per-element weight work is two DVE ops: tensor_scalar (4x perf mode:
(e08s2 * es1f[j]) max es02s1[j], per-partition f32 scalars) and a
tensor_tensor mask multiply against the PE-transposed adjacency (PSUM).

Host-side prep (cheap, O(N*IN_F) / O(N^2) casts): x shipped already
transposed ([k-part, j] layout, bf16) so h = x @ W needs no on-device
transposes; s1 = x@(W@a1), s2 = x@(W@a2) shipped as vectors; adj
re-encoded int8 (values 0/1; 4x less HBM traffic than int32).
int32 cast DMAs are catastrophic on HW (SWDGE emits per-element
descriptors, ~6 ns/elem), so adjacency is DMAed raw (HWDGE 2 KB
descriptors) and cast int8->bf16 on the otherwise-idle ACT engine.

The adjacency is shipped PRE-TRANSPOSED from the host ([j, i] layout,
int8, tiled so each partition line is contiguous), which removes all
on-device PE transposes and keeps every DVE operand in SBUF (2x tier):
Per-core pipeline over j-chunks jc of 128 (i in blocks b of 512):
    sync DMA: adjT int8 slab [128p=j, SBLK, ROWS=i]  (6 KB descriptors)
    ACT: cast slab slice -> adj_w [128, ROWS] bf16
    DVE: ta2 = (e08s2b * es1f[jc]) max es02s1[jc]   [128, ROWS] bf16 (4x)
    DVE: wT = ta2 * adj_w   [128, ROWS] bf16 (2x, full width)
    PE: out2[b][f,i] += h[jc] @ wT[:, b] ; every rsum_group chunks:
        rsum[b] += ones @ (DVE group-sum of wT)   (psum accum)
The h = x @ W sweep (2 matmuls per block) is interleaved into the first
NB main-loop chunks so no engine serializes behind setup.
Finalize per i-block: reciprocal of rowsum, transpose back, scale, store.

Walrus codegen rejects instructions carrying more than one sync-wait
("Too many sync wait commands"), so after Tile scheduling we legalize the
program: excess waits are moved onto injected same-engine nop instructions
placed immediately before the over-constrained instruction.
"""

import copy
import sys
from contextlib import ExitStack

import numpy as np

if "/opt/trn_rl_repo" not in sys.path:
    sys.path.insert(0, "/opt/trn_rl_repo")

import concourse.bass as bass
import concourse.tile as tile
from concourse import mybir
from concourse.masks import make_identity

P = 128
N_CORES = 8

F32 = mybir.dt.float32
BF16 = mybir.dt.bfloat16
I32 = mybir.dt.int32
I8 = mybir.dt.int8
AX = mybir.AluOpType
AF = mybir.ActivationFunctionType

_WAIT_SPLIT_SKIP = {"InstHalt", "InstSemWait", "InstEventSemOp"}


def _legalize_waits(nc, template_nop):
    """Move excess sync-waits onto injected same-engine nops."""
    uid = 0
    for f in nc.m.functions:
        for b in f.blocks:
            new_list = []
            changed = False
            for inst in b.instructions:
                si = inst.sync_info
                if (si is not None and len(si.on_wait) > 1
                        and type(inst).__name__ not in _WAIT_SPLIT_SKIP):
                    waits = list(si.on_wait)
                    for w in waits[:-1]:
                        uid += 1
                        nop = copy.copy(template_nop)
                        nop.name = f"I-lwsplit-{uid}"
                        nop.engine = inst.engine
                        nop.sync_info = mybir.SyncInfo(
                            on_wait=[w], on_update=[])
                        try:
                            nop.set_dependency_edges([])
                        except Exception:
                            pass
                        new_list.append(nop)
                    inst.sync_info = mybir.SyncInfo(
                        on_wait=[waits[-1]], on_update=list(si.on_update))
                    changed = True
                new_list.append(inst)
            if changed:
                b.instructions = new_list


def build_program(N=12288, IN_F=256, OUT_F=128, alpha=0.2, legalize=True,
                  sblk=4, tr_bufs=2, ta_bufs=4, wt_bufs=4, bf_bufs=3,
                  raw_bufs=2, xc=24, rsum_group=1, convert_on="act",
                  host_h=True, probe=()):
    """Single-core SPMD program. Per-core inputs:
      adjt_s [P, NB*ROWS] i8 (own columns of adj, transposed+tiled:
          [p, s, i] = adj[row0+i, s*128+p], 0/1),
      xtt [P, KB*NB*P] bf16 (x transposed: [p, k2, c, q] = x[c*128+q,
          k2*128+p]),
      wx [IN_F, OUT_F] f32 (W),
      s1t [P, NB] f32 (s1[c*128+p] at [p, c]),
      s2r [1, ROWS] f32 (s2 of own rows).
    Output [ROWS, OUT_F] f32.
    """
    ROWS = N // N_CORES
    NB = N // P
    KB = IN_F // P
    RB = ROWS // P
    IBS = 512 if ROWS % 512 == 0 else P
    IB = ROWS // IBS
    SUBS = IBS // P
    SBLK = sblk if NB % sblk == 0 else 1
    JCC = NB // SBLK
    XC = xc if NB % xc == 0 else NB
    TRW = max(IBS, 2 * OUT_F)
    g = rsum_group

    nc = bass.Bass(trn_type="TRN2")
    adjt_s = nc.dram_tensor("adjt_s", [P, NB * ROWS], I8,
                            kind="ExternalInput")
    if host_h:
        ht_d = nc.dram_tensor("ht", [P, NB * OUT_F], BF16,
                              kind="ExternalInput")
    else:
        xtt_d = nc.dram_tensor("xtt", [P, KB * NB * P], BF16,
                               kind="ExternalInput")
        wx_d = nc.dram_tensor("wx", [IN_F, OUT_F], F32,
                              kind="ExternalInput")
    s1t_d = nc.dram_tensor("s1t", [P, NB], F32, kind="ExternalInput")
    e08t_d = nc.dram_tensor("e08t", [P, ROWS], BF16, kind="ExternalInput")
    out_d = nc.dram_tensor("out", [ROWS, OUT_F], F32, kind="ExternalOutput")

    xtt_v = (None if host_h else
             xtt_d[:, :].rearrange("p (k c q) -> p k c q", k=KB, q=P))

    with tile.TileContext(nc) as tc, ExitStack() as ctx:
        template_nop = nc.sync.nop(nofuse=True).ins

        const = ctx.enter_context(tc.tile_pool(name="const", bufs=1))
        ident_f = const.tile([P, P], F32)
        make_identity(nc, ident_f[:])
        ident_b = const.tile([P, P], BF16)
        make_identity(nc, ident_b[:])
        ones_b = const.tile([P, 1], BF16)
        nc.gpsimd.memset(ones_b[:], 1.0)
        ones1_f = const.tile([1, P], F32)
        nc.gpsimd.memset(ones1_f[:], 1.0)

        h_sb = const.tile([P, NB * OUT_F], BF16)
        s1_sb = const.tile([P, NB], F32)
        es1f_sb = const.tile([P, NB], F32)   # exp(s1[j])
        es02_sb = const.tile([P, NB], F32)   # exp(0.2*s1[j])
        e08s2b = const.tile([P, ROWS], BF16)  # exp(0.8*s2[i]) bcast
        wxb = const.tile([P, KB * OUT_F], BF16)

        # ---------------- light setup ----------------
        su_sb = ctx.enter_context(tc.tile_pool(name="su_sb", bufs=2))
        su_ps = ctx.enter_context(tc.tile_pool(name="su_ps", bufs=1,
                                               space="PSUM"))

        if host_h:
            nc.scalar.dma_start(h_sb[:], ht_d[:, :])
        else:
            wxf = su_sb.tile([P, KB, OUT_F], F32, tag="wxf")
            nc.scalar.dma_start(
                wxf[:], wx_d[:, :].rearrange("(c p) f -> p c f", p=P))
            nc.vector.tensor_copy(wxb[:],
                                  wxf[:].rearrange("p c f -> p (c f)"))

        nc.scalar.dma_start(s1_sb[:], s1t_d[:, :])
        for q in range(0, NB, 24):
            w = min(24, NB - q)
            nc.scalar.activation(es1f_sb[:, q:q + w], s1_sb[:, q:q + w],
                                 AF.Exp)
            nc.scalar.activation(es02_sb[:, q:q + w], s1_sb[:, q:q + w],
                                 AF.Exp, scale=alpha)

        nc.scalar.dma_start(e08s2b[:], e08t_d[:, :])

        # ---------------- main pools ----------------
        ps_out = ctx.enter_context(
            tc.tile_pool(name="ps_out", bufs=1, space="PSUM"))
        ps_rs = ctx.enter_context(
            tc.tile_pool(name="ps_rs", bufs=1, space="PSUM"))
        ps_tr = ctx.enter_context(
            tc.tile_pool(name="ps_tr", bufs=tr_bufs, space="PSUM"))
        raw_pool = ctx.enter_context(tc.tile_pool(name="adj_raw",
                                                  bufs=raw_bufs))
        bf_pool = ctx.enter_context(tc.tile_pool(name="adj_bf",
                                                 bufs=bf_bufs))
        ta_pool = ctx.enter_context(tc.tile_pool(name="ta", bufs=ta_bufs))
        wt_pool = ctx.enter_context(tc.tile_pool(name="wt", bufs=wt_bufs))
        wg_pool = ctx.enter_context(tc.tile_pool(name="wg", bufs=3))
        xs_pool = ctx.enter_context(tc.tile_pool(name="xs", bufs=2))
        fin_pool = ctx.enter_context(tc.tile_pool(name="fin", bufs=2))

        out2 = [ps_out.tile([P, IBS], F32, tag=f"o{b}", name=f"out2_{b}")
                for b in range(IB)]
        rs_all = ps_rs.tile([P, IBS], F32, name="rs_all")
        rsum = [rs_all[32 * b:32 * b + 1, :] for b in range(IB)]

        xstage = [None]

        def x_block_pair(jb):
            # Two blocks of the h = x @ W sweep, interleaved into the main
            # loop so no engine serializes behind a monolithic setup.
            if jb % XC == 0:
                xstage[0] = xs_pool.tile([P, KB, XC, P], BF16, tag="xst",
                                         name=f"xst_{jb // XC}")
                nc.scalar.dma_start(
                    xstage[0][:], xtt_v[:, :, jb:jb + XC, :])
            h_ps = su_ps.tile([P, 2, OUT_F], F32, tag="h", name=f"h_{jb}")
            for u in range(2):
                for k2 in range(KB):
                    nc.tensor.matmul(
                        h_ps[:, u, :],
                        xstage[0][:, k2, (jb + u) % XC, :],
                        wxb[:, k2 * OUT_F:(k2 + 1) * OUT_F],
                        start=(k2 == 0), stop=(k2 == KB - 1))
            nc.scalar.copy(
                h_sb[:, jb * OUT_F:(jb + 2) * OUT_F],
                h_ps[:].rearrange("p u f -> p (u f)"))

        pending = []
        rs_acc = [None]
        for jcc in range(JCC):
            adj_raw = raw_pool.tile([P, SBLK * ROWS], I8, tag="adj_raw",
                                    name=f"adjr_{jcc}")
            nc.sync.dma_start(
                adj_raw[:],
                adjt_s[:, jcc * SBLK * ROWS:(jcc + 1) * SBLK * ROWS])
            for js in range(SBLK):
                jc = jcc * SBLK + js
                first, last = jc == 0, jc == NB - 1
                if not host_h and jc < NB and jc % 2 == 0:
                    x_block_pair(jc)
                raw_sl = adj_raw[:, js * ROWS:(js + 1) * ROWS]
                if convert_on == "none":
                    adj_w = raw_sl
                else:
                    adj_wt = bf_pool.tile([P, ROWS], BF16, tag="adjb",
                                          name=f"adjb_{jc}")
                    if convert_on == "act" or (convert_on == "mix"
                                               and jc % 2 == 0):
                        nc.scalar.copy(adj_wt[:], raw_sl)
                    else:
                        nc.vector.tensor_copy(adj_wt[:], raw_sl)
                    adj_w = adj_wt[:]
                ta2 = ta_pool.tile([P, ROWS], BF16, tag="ta2",
                                   name=f"ta2_{jc}")
                nc.vector.tensor_scalar(
                    ta2[:], e08s2b[:], es1f_sb[:, jc:jc + 1],
                    es02_sb[:, jc:jc + 1], op0=AX.mult, op1=AX.max)
                wT = wt_pool.tile([P, ROWS], BF16, tag="wT",
                                  name=f"wT_{jc}")
                nc.vector.tensor_mul(wT[:], ta2[:], adj_w)
                if g > 1:
                    if jc % g == 0:
                        rs_acc[0] = wT
                    else:
                        acc = wg_pool.tile([P, ROWS], BF16, tag="wg",
                                           name=f"wg_{jc}")
                        nc.vector.tensor_add(acc[:], rs_acc[0][:], wT[:])
                        rs_acc[0] = acc
                # software-pipeline the PE stream one stage: this chunk's
                # matmuls are emitted after the NEXT chunk's DVE ops.
                pending.append((jc, wT, first, last))
                if len(pending) > 1:
                    pjc, pw, pfirst, plast = pending.pop(0)
                    for b in range(IB):
                        nc.tensor.matmul(
                            out2[b][:],
                            h_sb[:, pjc * OUT_F:(pjc + 1) * OUT_F],
                            pw[:, b * IBS:(b + 1) * IBS],
                            start=pfirst, stop=plast)
                    if g == 1:
                        for b in range(IB):
                            nc.tensor.matmul(rsum[b], ones_b[:],
                                             pw[:, b * IBS:(b + 1) * IBS],
                                             start=pfirst, stop=plast)
                if g > 1 and (jc % g == g - 1 or last):
                    for b in range(IB):
                        nc.tensor.matmul(rsum[b], ones_b[:],
                                         rs_acc[0][:, b * IBS:(b + 1) * IBS],
                                         start=jc < g, stop=last)

        while pending:
            pjc, pw, pfirst, plast = pending.pop(0)
            for b in range(IB):
                nc.tensor.matmul(
                    out2[b][:], h_sb[:, pjc * OUT_F:(pjc + 1) * OUT_F],
                    pw[:, b * IBS:(b + 1) * IBS],
                    start=pfirst, stop=plast)
            if g == 1:
                for b in range(IB):
                    nc.tensor.matmul(rsum[b], ones_b[:],
                                     pw[:, b * IBS:(b + 1) * IBS],
                                     start=pfirst, stop=plast)

        # ---------------- finalize ----------------
        for b in range(IB):
            o_sb = fin_pool.tile([P, IBS], F32, tag="osb")
            nc.vector.tensor_copy(o_sb[:], out2[b][:])
            rs_sb = fin_pool.tile([1, IBS], F32, tag="rssb")
            nc.vector.tensor_copy(rs_sb[:], rsum[b])
            rall = fin_pool.tile([P, SUBS], F32, tag="rall")
            for t in range(SUBS):
                rT_ps = ps_tr.tile([P, TRW], BF16, tag="tr",
                                   name=f"rT_{b}_{t}")
                rT = rT_ps[:, 0:2].bitcast(F32)
                nc.tensor.matmul(rT[:, 0:1], rs_sb[0:1, t * P:(t + 1) * P],
                                 ones1_f[0:1, 0:1], start=True, stop=True)
                nc.vector.tensor_copy(rall[:, t:t + 1], rT[:, 0:1])
            rinv = fin_pool.tile([P, SUBS], F32, tag="rinv")
            nc.vector.reciprocal(rinv[:], rall[:])
            for t in range(SUBS):
                oT_ps = ps_tr.tile([P, TRW], BF16, tag="tr",
                                   name=f"oT_{b}_{t}")
                oT = oT_ps[:, 0:2 * OUT_F].bitcast(F32)
                nc.tensor.transpose(oT[:], o_sb[:, t * P:(t + 1) * P],
                                    ident_f[:])
                fin = fin_pool.tile([P, OUT_F], F32, tag="fint")
                nc.vector.tensor_scalar_mul(fin[:], oT[:, :OUT_F],
                                            rinv[:, t:t + 1])
                nc.scalar.dma_start(
                    out_d[b * IBS + t * P:b * IBS + (t + 1) * P, :], fin[:])

    if legalize:
        _legalize_waits(nc, template_nop)
    return nc


_PROG_CACHE = {}


def _get_program(N, IN_F, OUT_F):
    key = (N, IN_F, OUT_F)
    if key not in _PROG_CACHE:
        _PROG_CACHE[key] = build_program(N, IN_F, OUT_F)
    return _PROG_CACHE[key]


def make_in_maps(x, adj, W, a1, a2):
    import ml_dtypes
    bf16 = ml_dtypes.bfloat16
    N, IN_F = x.shape
    ROWS = N // N_CORES
    NB = N // P
    KB = IN_F // P
    ROWS = N // N_CORES
    x = np.asarray(x, dtype=np.float32)
    W = np.asarray(W, dtype=np.float32)
    wx = np.ascontiguousarray(W)
    s1 = (x @ (W @ np.asarray(a1, dtype=np.float32))).astype(np.float32)
    s2 = (x @ (W @ np.asarray(a2, dtype=np.float32))).astype(np.float32)
    ROWS_ = ROWS
    s1t = np.ascontiguousarray(s1.reshape(NB, P).T)
    # xtt[p, k2, c, q] = x[c*128+q, k2*128+p]
    xtt = np.ascontiguousarray(
        x.reshape(NB, P, KB, P).transpose(3, 2, 0, 1).astype(bf16)
    ).reshape(P, KB * NB * P)
    h = (x @ W).astype(np.float32)
    OUT_F_ = W.shape[1]
    ht = np.ascontiguousarray(
        h.reshape(NB, P, OUT_F_).transpose(1, 0, 2).astype(bf16)
    ).reshape(P, NB * OUT_F_)
    adj8 = np.asarray(adj, dtype=np.int8)
    in_maps = []
    for c in range(N_CORES):
        sl = slice(c * ROWS, (c + 1) * ROWS)
        # adjt[p, s, i] = adj[row0+i, s*128+p]
        adjt = np.ascontiguousarray(
            adj8[sl].T.reshape(NB, P, ROWS).transpose(1, 0, 2)
        ).reshape(P, NB * ROWS)
        in_maps.append({
            "adjt_s": adjt,
            "xtt": xtt,
            "wx": wx,
            "ht": ht,
            "s1t": s1t,
            "e08t": np.ascontiguousarray(np.broadcast_to(
                np.exp(0.8 * s2[sl].astype(np.float64)).astype(bf16)[None, :],
                (P, ROWS_))),
        })
    return in_maps


def kernel(x, adj, W, a1, a2, trace=False):
    x = np.asarray(x, dtype=np.float32)
    W = np.asarray(W, dtype=np.float32)
    a1 = np.asarray(a1, dtype=np.float32)
    a2 = np.asarray(a2, dtype=np.float32)
    N, IN_F = x.shape
    OUT_F = W.shape[1]

    from concourse.bass_utils import run_bass_kernel_spmd

    nc = _get_program(N, IN_F, OUT_F)
    in_maps = make_in_maps(x, adj, W, a1, a2)
    res = run_bass_kernel_spmd(
        nc, in_maps, core_ids=list(range(N_CORES)), trace=trace)
    out = np.concatenate([r["out"] for r in res.results], axis=0)
    kernel.last_results = res
    return out


# revision 18
# speedup vs baseline: 2.4967x; 2.4967x over previous
"""GAT-style attention kernel for Trainium2, 8-core row-parallel.

Reference computation:
    h = x @ W; s1 = h @ a1; s2 = h @ a2
    e[i,j] = leaky_relu(s1[j] + s2[i], 0.2); masked by adj; row-softmax; @ h

Key algebraic trick: with the column rescale w~ = w / exp(0.2*s2[i]),
    w~[j,i] = adj[i,j] * max(exp(s1[j] + 0.8*s2[i]), exp(0.2*s1[j]))
and the rescale cancels in the softmax normalization:
    out[i,:] = (sum_j w~[j,i] h[j,:]) / (sum_j w~[j,i]).
The exp is separable: exp(s1[j] + 0.8*s2[i]) = exp(s1[j]) * exp(0.8*s2[i]),
so per-element weight work is two DVE ops: tensor_scalar (4x perf mode:
(e08s2 * es1f[j]) max es02s1[j], per-partition f32 scalars) and a
tensor_tensor mask multiply against the PE-transposed adjacency (PSUM).

Host-side prep (cheap, O(N*IN_F) / O(N^2) casts): x shipped already
transposed ([k-part, j] layout, bf16) so h = x @ W needs no on-device
transposes; s1 = x@(W@a1), s2 = x@(W@a2) shipped as vectors; adj
re-encoded int8 (values 0/1; 4x less HBM traffic than int32).
int32 cast DMAs are catastrophic on HW (SWDGE emits per-element
descriptors, ~6 ns/elem), so adjacency is DMAed raw (HWDGE 2 KB
descriptors) and cast int8->bf16 on the otherwise-idle ACT engine.

The adjacency is shipped PRE-TRANSPOSED from the host ([j, i] layout,
int8, tiled so each partition line is contiguous), which removes all
on-device PE transposes and keeps every DVE operand in SBUF (2x tier):
Per-core pipeline over j-chunks jc of 128 (i in blocks b of 512):
    sync DMA: adjT int8 slab [128p=j, SBLK, ROWS=i]  (6 KB descriptors)
    ACT: cast slab slice -> adj_w [128, ROWS] bf16
    DVE: ta2 = (e08s2b * es1f[jc]) max es02s1[jc]   [128, ROWS] bf16 (4x)
    DVE: wT = ta2 * adj_w   [128, ROWS] bf16 (2x, full width)
    PE: out2[b][f,i] += h[jc] @ wT[:, b] ; every rsum_group chunks:
        rsum[b] += ones @ (DVE group-sum of wT)   (psum accum)
The h = x @ W sweep (2 matmuls per block) is interleaved into the first
NB main-loop chunks so no engine serializes behind setup.
Finalize per i-block: reciprocal of rowsum, transpose back, scale, store.

Walrus codegen rejects instructions carrying more than one sync-wait
("Too many sync wait commands"), so after Tile scheduling we legalize the
program: excess waits are moved onto injected same-engine nop instructions
placed immediately before the over-constrained instruction.
"""

import copy
import sys
from contextlib import ExitStack

import numpy as np

if "/opt/trn_rl_repo" not in sys.path:
    sys.path.insert(0, "/opt/trn_rl_repo")

import concourse.bass as bass
import concourse.tile as tile
from concourse import mybir
from concourse.masks import make_identity

P = 128
N_CORES = 8

F32 = mybir.dt.float32
BF16 = mybir.dt.bfloat16
I32 = mybir.dt.int32
I8 = mybir.dt.int8
AX = mybir.AluOpType
AF = mybir.ActivationFunctionType

_WAIT_SPLIT_SKIP = {"InstHalt", "InstSemWait", "InstEventSemOp"}


def _legalize_waits(nc, template_nop):
    """Move excess sync-waits onto injected same-engine nops."""
    uid = 0
    for f in nc.m.functions:
        for b in f.blocks:
            new_list = []
            changed = False
            for inst in b.instructions:
                si = inst.sync_info
                if (si is not None and len(si.on_wait) > 1
                        and type(inst).__name__ not in _WAIT_SPLIT_SKIP):
                    waits = list(si.on_wait)
                    for w in waits[:-1]:
                        uid += 1
                        nop = copy.copy(template_nop)
                        nop.name = f"I-lwsplit-{uid}"
                        nop.engine = inst.engine
                        nop.sync_info = mybir.SyncInfo(
                            on_wait=[w], on_update=[])
                        try:
                            nop.set_dependency_edges([])
                        except Exception:
                            pass
                        new_list.append(nop)
                    inst.sync_info = mybir.SyncInfo(
                        on_wait=[waits[-1]], on_update=list(si.on_update))
                    changed = True
                new_list.append(inst)
            if changed:
                b.instructions = new_list


def build_program(N=12288, IN_F=256, OUT_F=128, alpha=0.2, legalize=True,
                  sblk=4, tr_bufs=2, ta_bufs=4, wt_bufs=4, bf_bufs=3,
                  raw_bufs=2, xc=24, rsum_group=1, convert_on="act",
                  host_h=True, probe=()):
    """Single-core SPMD program. Per-core inputs:
      adjt_s [P, NB*ROWS] i8 (own columns of adj, transposed+tiled:
          [p, s, i] = adj[row0+i, s*128+p], 0/1),
      xtt [P, KB*NB*P] bf16 (x transposed: [p, k2, c, q] = x[c*128+q,
          k2*128+p]),
      wx [IN_F, OUT_F] f32 (W),
      s1t [P, NB] f32 (s1[c*128+p] at [p, c]),
      s2r [1, ROWS] f32 (s2 of own rows).
    Output [ROWS, OUT_F] f32.
    """
    ROWS = N // N_CORES
    NB = N // P
    KB = IN_F // P
    RB = ROWS // P
    IBS = 512 if ROWS % 512 == 0 else P
    IB = ROWS // IBS
    SUBS = IBS // P
    SBLK = sblk if NB % sblk == 0 else 1
    JCC = NB // SBLK
    XC = xc if NB % xc == 0 else NB
    TRW = max(IBS, 2 * OUT_F)
    g = rsum_group

    nc = bass.Bass(trn_type="TRN2")
    adjt_s = nc.dram_tensor("adjt_s", [P, NB * ROWS], I8,
                            kind="ExternalInput")
    if host_h:
        ht_d = nc.dram_tensor("ht", [P, NB * OUT_F], BF16,
                              kind="ExternalInput")
    else:
        xtt_d = nc.dram_tensor("xtt", [P, KB * NB * P], BF16,
                               kind="ExternalInput")
        wx_d = nc.dram_tensor("wx", [IN_F, OUT_F], F32,
                              kind="ExternalInput")
    s1t_d = nc.dram_tensor("s1t", [P, NB], F32, kind="ExternalInput")
    e08t_d = nc.dram_tensor("e08t", [P, ROWS], BF16, kind="ExternalInput")
    out_d = nc.dram_tensor("out", [ROWS, OUT_F], F32, kind="ExternalOutput")

    xtt_v = (None if host_h else
             xtt_d[:, :].rearrange("p (k c q) -> p k c q", k=KB, q=P))

    with tile.TileContext(nc) as tc, ExitStack() as ctx:
        template_nop = nc.sync.nop(nofuse=True).ins

        const = ctx.enter_context(tc.tile_pool(name="const", bufs=1))
        ident_f = const.tile([P, P], F32)
        make_identity(nc, ident_f[:])
        ident_b = const.tile([P, P], BF16)
        make_identity(nc, ident_b[:])
        ones_b = const.tile([P, 1], BF16)
        nc.gpsimd.memset(ones_b[:], 1.0)
        ones1_f = const.tile([1, P], F32)
        nc.gpsimd.memset(ones1_f[:], 1.0)

        h_sb = const.tile([P, NB * OUT_F], BF16)
        s1_sb = const.tile([P, NB], F32)
        es1f_sb = const.tile([P, NB], F32)   # exp(s1[j])
        es02_sb = const.tile([P, NB], F32)   # exp(0.2*s1[j])
        e08s2b = const.tile([P, ROWS], BF16)  # exp(0.8*s2[i]) bcast
        wxb = const.tile([P, KB * OUT_F], BF16)

        # ---------------- light setup ----------------
        su_sb = ctx.enter_context(tc.tile_pool(name="su_sb", bufs=2))
        su_ps = ctx.enter_context(tc.tile_pool(name="su_ps", bufs=1,
                                               space="PSUM"))

        if host_h:
            nc.scalar.dma_start(h_sb[:], ht_d[:, :])
        else:
            wxf = su_sb.tile([P, KB, OUT_F], F32, tag="wxf")
            nc.scalar.dma_start(
                wxf[:], wx_d[:, :].rearrange("(c p) f -> p c f", p=P))
            nc.vector.tensor_copy(wxb[:],
                                  wxf[:].rearrange("p c f -> p (c f)"))

        nc.scalar.dma_start(s1_sb[:], s1t_d[:, :])
        for q in range(0, NB, 24):
            w = min(24, NB - q)
            nc.scalar.activation(es1f_sb[:, q:q + w], s1_sb[:, q:q + w],
                                 AF.Exp)
            nc.scalar.activation(es02_sb[:, q:q + w], s1_sb[:, q:q + w],
                                 AF.Exp, scale=alpha)

        nc.scalar.dma_start(e08s2b[:], e08t_d[:, :])

        # ---------------- main pools ----------------
        ps_out = ctx.enter_context(
            tc.tile_pool(name="ps_out", bufs=1, space="PSUM"))
        ps_rs = ctx.enter_context(
            tc.tile_pool(name="ps_rs", bufs=1, space="PSUM"))
        ps_tr = ctx.enter_context(
            tc.tile_pool(name="ps_tr", bufs=tr_bufs, space="PSUM"))
        raw_pool = ctx.enter_context(tc.tile_pool(name="adj_raw",
                                                  bufs=raw_bufs))
        bf_pool = ctx.enter_context(tc.tile_pool(name="adj_bf",
                                                 bufs=bf_bufs))
        ta_pool = ctx.enter_context(tc.tile_pool(name="ta", bufs=ta_bufs))
        wt_pool = ctx.enter_context(tc.tile_pool(name="wt", bufs=wt_bufs))
        wg_pool = ctx.enter_context(tc.tile_pool(name="wg", bufs=3))
        xs_pool = ctx.enter_context(tc.tile_pool(name="xs", bufs=2))
        fin_pool = ctx.enter_context(tc.tile_pool(name="fin", bufs=2))

        out2 = [ps_out.tile([P, IBS], F32, tag=f"o{b}", name=f"out2_{b}")
                for b in range(IB)]
        rs_all = ps_rs.tile([P, IBS], F32, name="rs_all")
        rsum = [rs_all[32 * b:32 * b + 1, :] for b in range(IB)]

        xstage = [None]

        def x_block_pair(jb):
            # Two blocks of the h = x @ W sweep, interleaved into the main
            # loop so no engine serializes behind a monolithic setup.
            if jb % XC == 0:
                xstage[0] = xs_pool.tile([P, KB, XC, P], BF16, tag="xst",
                                         name=f"xst_{jb // XC}")
                nc.scalar.dma_start(
                    xstage[0][:], xtt_v[:, :, jb:jb + XC, :])
            h_ps = su_ps.tile([P, 2, OUT_F], F32, tag="h", name=f"h_{jb}")
            for u in range(2):
                for k2 in range(KB):
                    nc.tensor.matmul(
                        h_ps[:, u, :],
                        xstage[0][:, k2, (jb + u) % XC, :],
                        wxb[:, k2 * OUT_F:(k2 + 1) * OUT_F],
                        start=(k2 == 0), stop=(k2 == KB - 1))
            nc.scalar.copy(
                h_sb[:, jb * OUT_F:(jb + 2) * OUT_F],
                h_ps[:].rearrange("p u f -> p (u f)"))

        pending = []
        rs_acc = [None]
        for jcc in range(JCC):
            adj_raw = raw_pool.tile([P, SBLK * ROWS], I8, tag="adj_raw",
                                    name=f"adjr_{jcc}")
            nc.sync.dma_start(
                adj_raw[:],
                adjt_s[:, jcc * SBLK * ROWS:(jcc + 1) * SBLK * ROWS])
            for js in range(SBLK):
                jc = jcc * SBLK + js
                first, last = jc == 0, jc == NB - 1
                if not host_h and jc < NB and jc % 2 == 0:
                    x_block_pair(jc)
                raw_sl = adj_raw[:, js * ROWS:(js + 1) * ROWS]
                if convert_on == "none":
                    adj_w = raw_sl
                else:
                    adj_wt = bf_pool.tile([P, ROWS], BF16, tag="adjb",
                                          name=f"adjb_{jc}")
                    if convert_on == "act" or (convert_on == "mix"
                                               and jc % 2 == 0):
                        nc.scalar.copy(adj_wt[:], raw_sl)
                    else:
                        nc.vector.tensor_copy(adj_wt[:], raw_sl)
                    adj_w = adj_wt[:]
                ta2 = ta_pool.tile([P, ROWS], BF16, tag="ta2",
                                   name=f"ta2_{jc}")
                nc.vector.tensor_scalar(
                    ta2[:], e08s2b[:], es1f_sb[:, jc:jc + 1],
                    es02_sb[:, jc:jc + 1], op0=AX.mult, op1=AX.max)
                wT = wt_pool.tile([P, ROWS], BF16, tag="wT",
                                  name=f"wT_{jc}")
                nc.vector.tensor_mul(wT[:], ta2[:], adj_w)
                if g > 1:
                    if jc % g == 0:
                        rs_acc[0] = wT
                    else:
                        acc = wg_pool.tile([P, ROWS], BF16, tag="wg",
                                           name=f"wg_{jc}")
                        nc.vector.tensor_add(acc[:], rs_acc[0][:], wT[:])
                        rs_acc[0] = acc
                # software-pipeline the PE stream one stage: this chunk's
                # matmuls are emitted after the NEXT chunk's DVE ops.
                pending.append((jc, wT, first, last))
                if len(pending) > 1:
                    pjc, pw, pfirst, plast = pending.pop(0)
                    for b in range(IB):
                        nc.tensor.matmul(
                            out2[b][:],
                            h_sb[:, pjc * OUT_F:(pjc + 1) * OUT_F],
                            pw[:, b * IBS:(b + 1) * IBS],
                            start=pfirst, stop=plast)
                    if g == 1:
                        for b in range(IB):
                            nc.tensor.matmul(rsum[b], ones_b[:],
                                             pw[:, b * IBS:(b + 1) * IBS],
                                             start=pfirst, stop=plast)
                if g > 1 and (jc % g == g - 1 or last):
                    for b in range(IB):
                        nc.tensor.matmul(rsum[b], ones_b[:],
                                         rs_acc[0][:, b * IBS:(b + 1) * IBS],
                                         start=jc < g, stop=last)

        while pending:
            pjc, pw, pfirst, plast = pending.pop(0)
            for b in range(IB):
                nc.tensor.matmul(
                    out2[b][:], h_sb[:, pjc * OUT_F:(pjc + 1) * OUT_F],
                    pw[:, b * IBS:(b + 1) * IBS],
                    start=pfirst, stop=plast)
            if g == 1:
                for b in range(IB):
                    nc.tensor.matmul(rsum[b], ones_b[:],
                                     pw[:, b * IBS:(b + 1) * IBS],
                                     start=pfirst, stop=plast)

        # ---------------- finalize ----------------
        for b in range(IB):
            o_sb = fin_pool.tile([P, IBS], F32, tag="osb")
            nc.vector.tensor_copy(o_sb[:], out2[b][:])
            rs_sb = fin_pool.tile([1, IBS], F32, tag="rssb")
            nc.vector.tensor_copy(rs_sb[:], rsum[b])
            rall = fin_pool.tile([P, SUBS], F32, tag="rall")
            for t in range(SUBS):
                rT_ps = ps_tr.tile([P, TRW], BF16, tag="tr",
                                   name=f"rT_{b}_{t}")
                rT = rT_ps[:, 0:2].bitcast(F32)
                nc.tensor.matmul(rT[:, 0:1], rs_sb[0:1, t * P:(t + 1) * P],
                                 ones1_f[0:1, 0:1], start=True, stop=True)
                nc.vector.tensor_copy(rall[:, t:t + 1], rT[:, 0:1])
            rinv = fin_pool.tile([P, SUBS], F32, tag="rinv")
            nc.vector.reciprocal(rinv[:], rall[:])
            for t in range(SUBS):
                oT_ps = ps_tr.tile([P, TRW], BF16, tag="tr",
                                   name=f"oT_{b}_{t}")
                oT = oT_ps[:, 0:2 * OUT_F].bitcast(F32)
                nc.tensor.transpose(oT[:], o_sb[:, t * P:(t + 1) * P],
                                    ident_f[:])
                fin = fin_pool.tile([P, OUT_F], F32, tag="fint")
                nc.vector.tensor_scalar_mul(fin[:], oT[:, :OUT_F],
                                            rinv[:, t:t + 1])
                nc.scalar.dma_start(
                    out_d[b * IBS + t * P:b * IBS + (t + 1) * P, :], fin[:])

    if legalize:
        _legalize_waits(nc, template_nop)
    return nc


_PROG_CACHE = {}


def _get_program(N, IN_F, OUT_F):
    key = (N, IN_F, OUT_F)
    if key not in _PROG_CACHE:
        _PROG_CACHE[key] = build_program(N, IN_F, OUT_F)
    return _PROG_CACHE[key]


def make_in_maps(x, adj, W, a1, a2):
    import ml_dtypes
    bf16 = ml_dtypes.bfloat16
    N, IN_F = x.shape
    ROWS = N // N_CORES
    NB = N // P
    KB = IN_F // P
    ROWS = N // N_CORES
    x = np.asarray(x, dtype=np.float32)
    W = np.asarray(W, dtype=np.float32)
    wx = np.ascontiguousarray(W)
    s1 = (x @ (W @ np.asarray(a1, dtype=np.float32))).astype(np.float32)
    s2 = (x @ (W @ np.asarray(a2, dtype=np.float32))).astype(np.float32)
    ROWS_ = ROWS
    s1t = np.ascontiguousarray(s1.reshape(NB, P).T)
    # xtt[p, k2, c, q] = x[c*128+q, k2*128+p]
    xtt = np.ascontiguousarray(
        x.reshape(NB, P, KB, P).transpose(3, 2, 0, 1).astype(bf16)
    ).reshape(P, KB * NB * P)
    h = (x @ W).astype(np.float32)
    OUT_F_ = W.shape[1]
    ht = np.ascontiguousarray(
        h.reshape(NB, P, OUT_F_).transpose(1, 0, 2).astype(bf16)
    ).reshape(P, NB * OUT_F_)
    adj8 = np.asarray(adj, dtype=np.int8)
    in_maps = []
    for c in range(N_CORES):
        sl = slice(c * ROWS, (c + 1) * ROWS)
        # adjt[p, s, i] = adj[row0+i, s*128+p]
        adjt = np.ascontiguousarray(
            adj8[sl].T.reshape(NB, P, ROWS).transpose(1, 0, 2)
        ).reshape(P, NB * ROWS)
        in_maps.append({
            "adjt_s": adjt,
            "xtt": xtt,
            "wx": wx,
            "ht": ht,
            "s1t": s1t,
            "e08t": np.ascontiguousarray(np.broadcast_to(
                np.exp(0.8 * s2[sl].astype(np.float64)).astype(bf16)[None, :],
                (P, ROWS_))),
        })
    return in_maps


def kernel(x, adj, W, a1, a2, trace=False):
    x = np.asarray(x, dtype=np.float32)
    W = np.asarray(W, dtype=np.float32)
    a1 = np.asarray(a1, dtype=np.float32)
    a2 = np.asarray(a2, dtype=np.float32)
    N, IN_F = x.shape
    OUT_F = W.shape[1]

    from concourse.bass_utils import run_bass_kernel_spmd

    nc = _get_program(N, IN_F, OUT_F)
    in_maps = make_in_maps(x, adj, W, a1, a2)
    names = {ml.memorylocations[0].name
             for ml in nc.m.functions[0].allocations
             if isinstance(ml, mybir.MemoryLocationSet)
             and ml.kind == "ExternalInput"}
    in_maps = [{k: v for k, v in m.items() if k in names} for m in in_maps]
    res = run_bass_kernel_spmd(
        nc, in_maps, core_ids=list(range(N_CORES)), trace=trace)
    out = np.concatenate([r["out"] for r in res.results], axis=0)
    kernel.last_results = res
    return out
